# revision 28
# baseline (speedup 1.0000x reference)
"""DeepseekV3 mini MoE MLP on 8 TRN2 NeuronCores.

Strategy: expert-parallel. The router (tiny: 0.1% of FLOPs) is computed
with jax ops that mirror the reference bit-for-bit; tokens are then
dispatched on the host to per-expert batches (the "all-to-all"), one
expert per NeuronCore. Each core runs a fused gate/up/silu/mul/down
kernel over its routed tokens. The combine (scatter-add weighted by the
top-k routing weights) happens on the host.

Matmul operands are fp16 (PE upconverts to FP22 internally, same
throughput as f32r but half the DMA traffic and SBUF footprint, and
fast-weight-load applies). PSUM accumulation stays fp32.

Layouts are feature-major ([dim, tokens]) so every matmul contracts over
the SBUF partition dim with no transposes anywhere on device. All DMAs
are shaped to be descriptor-friendly: x is packed on the host per token
tile as the exact SBUF image (128 fully-contiguous rows per transfer),
weights as [k_in(P), m_blk, k_blk, m_in] streamed in h-block-pair slabs.
Weight slabs + y writeback ride the Activation HWDGE ring; x tiles get
the SP ring to themselves. The down phase is software-pipelined one
token tile behind gate/up, so down weights aren't needed until ~75us in
and down matmuls never wait on the silu/mult chain. A few warm-up
matmuls on zeroed SBUF cover the first DMA's latency so the PE clock
(HAM) is already ramping before real work arrives.
"""

import numpy as np

import concourse.bass as bass
import concourse.mybir as mybir
import concourse.tile as tile
from concourse import bacc
from concourse.bass_utils import run_bass_kernel_spmd

DIM = 1024
HIDDEN = 1024
NUM_EXPERTS = 8
TOP_K = 2
P = 128
TT = 512  # main token tile (PSUM bank = 512 fp32)
DT = DIM // P  # 8 d-tiles
HT = HIDDEN // P  # 8 h-tiles
SW = 2  # h-blocks covered by the starter weight slab

F32 = mybir.dt.float32
F16 = mybir.dt.float16

_program_cache: dict[tuple, object] = {}
LAST_RESULT = None


def _build_expert_program(tiles: tuple):
    """One-expert FFN: yt = ((silu(x@wg.T) * (x@wu.T)) @ wd.T).T over C tokens.

    DRAM params (per core), all fp16:
      xt [DIM, C]          tokens, transposed (d-major)
      wg/wu [P, HT*DT*P]   gate/up proj packed [k_in, m_blk, k_blk, m_in]
      wd [P, HT*HT*P]      down proj packed    [k_in, m_blk, k_blk, m_in]
      yt [DIM, C]          output, transposed
    """
    C = sum(tiles)
    nc = bacc.Bacc(None, target_bir_lowering=False, debug=False)
    # xt is packed on the host per token tile as [half][p][a][token] — the
    # exact SBUF image — so every x DMA is 128 fully-contiguous descriptors.
    xt = nc.declare_dram_parameter("xt", [DIM * C], F16, isOutput=False)
    wg = nc.declare_dram_parameter("wg", [P, HT * DT * P], F16, isOutput=False)
    wu = nc.declare_dram_parameter("wu", [P, HT * DT * P], F16, isOutput=False)
    wd = nc.declare_dram_parameter("wd", [P, HT * HT * P], F16, isOutput=False)
    yt = nc.declare_dram_parameter("yt", [DIM, C], F16, isOutput=True)

    # gate/up weight slabs, streamed in consumption order. The first two are
    # single h-blocks (256KB) so the first matmuls' weights land ~1us sooner
    # and h0's up-weights arrive before its gate matmuls finish.
    WCUTS = (0, 1, 2, 4, 6, HT)
    NSCAL = 3  # slabs 0..2 ride the Activation ring, the rest ride SP

    with tile.TileContext(nc) as tc:
        with (
            tc.tile_pool(name="wpool", bufs=1) as wpool,
            tc.tile_pool(name="xpool", bufs=3) as xpool,
            tc.tile_pool(name="hpool", bufs=2) as hpool,
            tc.tile_pool(name="apool", bufs=3) as apool,
            tc.tile_pool(name="ypool", bufs=3) as ypool,
            tc.tile_pool(name="pg", bufs=2, space="PSUM") as pgpool,
            tc.tile_pool(name="pu", bufs=2, space="PSUM") as pupool,
            tc.tile_pool(name="py", bufs=2, space="PSUM") as pypool,
        ):
            wg_t, wu_t = [], []
            for i in range(len(WCUTS) - 1):
                n = (WCUTS[i + 1] - WCUTS[i]) * DT * P
                wg_t.append(wpool.tile([P, n], F16, name=f"wg{i}", tag=f"wg{i}"))
                wu_t.append(wpool.tile([P, n], F16, name=f"wu{i}", tag=f"wu{i}"))
            wd_a = wpool.tile([P, HT * HT * P], F16, name="wd_a", tag="wd_a")

            def w_at(tiles_, h, a):
                i = next(j for j in range(len(WCUTS) - 1) if h < WCUTS[j + 1])
                off = ((h - WCUTS[i]) * DT + a) * P
                return tiles_[i][:, off : off + P]

            # PE warm-up: a few matmuls on zeroed SBUF start the HAM busy
            # window while the first x/weight DMAs are still in flight.
            wz = wpool.tile([P, P], F16, name="wz", tag="wz")
            xz = wpool.tile([P, TT], F16, name="xz", tag="xz")
            nc.vector.memset(wz[:, :], 0.0)
            nc.vector.memset(xz[:, :], 0.0)
            pz = pgpool.tile([P, TT], F32, tag="pg")
            for _ in range(4):
                nc.tensor.matmul(pz[:, :], wz[:, :], xz[:, :], start=True, stop=True)

            XH = DT // 2  # d-blocks per x half-tile

            def load_x(t, tt, base, eng=None):
                eng = eng or nc.sync
                x_h = [
                    xpool.tile([P, XH * TT], F16, name=f"x{i}_{t}", tag=f"x{i}")
                    for i in range(2)
                ]
                for i in range(2):
                    lo = base + i * P * XH * tt
                    eng.dma_start(
                        out=x_h[i][:, :].rearrange("p (a t) -> p a t", a=XH)[:, :, :tt],
                        in_=xt.ap()[lo : lo + P * XH * tt].rearrange(
                            "(p a t) -> p a t", p=P, a=XH, t=tt
                        ),
                    )
                return x_h

            def gate_up(t, tt, x_h):
                h_sb = hpool.tile([P, HT * TT], F16, name=f"h_{t}", tag="h")
                for h in range(HT):
                    pg = pgpool.tile([P, tt], F32, tag="pg")
                    pu = pupool.tile([P, tt], F32, tag="pu")
                    for a in range(DT):
                        nc.tensor.matmul(
                            pg[:, :],
                            w_at(wg_t, h, a),
                            x_h[a // XH][:, (a % XH) * TT : (a % XH) * TT + tt],
                            start=(a == 0),
                            stop=(a == DT - 1),
                        )
                    for a in range(DT):
                        nc.tensor.matmul(
                            pu[:, :],
                            w_at(wu_t, h, a),
                            x_h[a // XH][:, (a % XH) * TT : (a % XH) * TT + tt],
                            start=(a == 0),
                            stop=(a == DT - 1),
                        )
                    act_sb = apool.tile([P, TT], F32, tag="act")
                    nc.scalar.activation(
                        act_sb[:, :tt], pg[:, :], mybir.ActivationFunctionType.Silu
                    )
                    nc.vector.tensor_tensor(
                        h_sb[:, h * TT : h * TT + tt],
                        act_sb[:, :tt],
                        pu[:, :],
                        mybir.AluOpType.mult,
                    )
                return h_sb

            def down(tt, ts, h_sb):
                for do in range(HT):
                    py = pypool.tile([P, tt], F32, tag="py")
                    for a in range(HT):
                        nc.tensor.matmul(
                            py[:, :],
                            wd_a[:, (do * HT + a) * P : (do * HT + a) * P + P],
                            h_sb[:, a * TT : a * TT + tt],
                            start=(a == 0),
                            stop=(a == HT - 1),
                        )
                    y_sb = ypool.tile([P, TT], F16, tag="y")
                    nc.vector.tensor_copy(y_sb[:, :tt], py[:, :])
                    nc.scalar.dma_start(
                        out=yt.ap()[do * P : (do + 1) * P, ts], in_=y_sb[:, :tt]
                    )

            # Software pipeline: gate/up for tile t, then down for tile t-1.
            # The down weights aren't needed until tile 1's gate/up is done
            # (~75us in), so the first-30us DMA rush is only x + gate/up
            # slabs. Down matmuls read an h_sb finished a full tile ago, so
            # they never wait on the silu/mult chain.
            prev = None  # (tt, ts, h_sb)
            x1_pre = None
            off = 0
            for t, tt in enumerate(tiles):
                ts = bass.ds(off, tt)
                if t == 1 and x1_pre is not None:
                    x_h = x1_pre
                else:
                    x_h = load_x(t, tt, DIM * off)
                off += tt
                if t == 0:
                    # Split the gate/up slab stream across both ~110GB/s
                    # rings so the last slab lands ~11us sooner: pairs 01/23
                    # on Activation, pairs 45/67 on SP behind x0. x1 rides
                    # Activation (SP is busy with the tail slabs); down
                    # weights follow on Activation — first needed ~75us in.
                    for i in range(len(WCUTS) - 1):
                        lo, hi = WCUTS[i] * DT * P, WCUTS[i + 1] * DT * P
                        eng = nc.scalar if i < NSCAL else nc.sync
                        eng.dma_start(out=wg_t[i][:, :], in_=wg.ap()[:, lo:hi])
                        eng.dma_start(out=wu_t[i][:, :], in_=wu.ap()[:, lo:hi])
                    if len(tiles) > 1:
                        x1_pre = load_x(1, tiles[1], DIM * off, eng=nc.scalar)
                    nc.scalar.dma_start(out=wd_a[:, :], in_=wd.ap()[:, :])
                h_sb = gate_up(t, tt, x_h)
                if prev is not None:
                    down(*prev)
                prev = (tt, ts, h_sb)
            down(*prev)
    nc.compile()
    return nc


def _tiles_for(max_cnt: int) -> tuple:
    """Token tiles covering max_cnt: a 384-token ramp tile (small so the
    first x DMA lands fast), full 512s, and a >=256 multiple-of-64 tail."""
    if max_cnt <= 384:
        return (max(256, ((max_cnt + 63) // 64) * 64),)
    rem = max_cnt - 384
    full, tail = divmod(rem, TT)
    tail = ((tail + 63) // 64) * 64
    if tail == 0:
        return (384,) + (TT,) * full
    if tail < 256:
        if full == 0:
            return (384, max(256, tail))
        full -= 1
        tail += TT  # in (512, 768): split into two >=256 tiles
        a = 256
        return (384,) + (TT,) * full + (a, tail - a)
    return (384,) + (TT,) * full + (tail,)


def _get_program(tiles: tuple):
    if tiles not in _program_cache:
        _program_cache[tiles] = _build_expert_program(tiles)
    return _program_cache[tiles]


def _pack_w(wt: np.ndarray) -> np.ndarray:
    """[K, M] weight (K contracted) -> [k_in(P), m_blk, k_blk, m_in] flattened
    to [P, M//P * K//P * P]; slab for output block mb is contiguous per row."""
    K, M = wt.shape
    return np.ascontiguousarray(
        wt.astype(np.float16)
        .reshape(K // P, P, M // P, P)
        .transpose(1, 2, 0, 3)
        .reshape(P, (M // P) * (K // P) * P)
    )


def _route(flat: np.ndarray, gate_w: np.ndarray):
    """Mirror the reference router bit-for-bit (jax ops, same backend)."""
    try:
        import jax
        import jax.numpy as jnp

        logits = jnp.asarray(flat) @ jnp.asarray(gate_w).T
        scores = jax.nn.sigmoid(logits)
        top_val, top_idx = jax.lax.top_k(scores, TOP_K)
        top_val = top_val / (top_val.sum(-1, keepdims=True) + 1e-9)
        return np.asarray(top_val), np.asarray(top_idx)
    except Exception:
        # numpy fallback: identical selection semantics (stable descending)
        logits = flat @ gate_w.T
        scores = 1.0 / (1.0 + np.exp(-logits))
        order = np.argsort(-scores, axis=-1, kind="stable")
        top_idx = order[:, :TOP_K].astype(np.int32)
        top_val = np.take_along_axis(scores, top_idx, axis=-1)
        top_val = top_val / (top_val.sum(-1, keepdims=True) + 1e-9)
        return top_val.astype(np.float32), top_idx


def kernel(x, gate_w, gate_proj, up_proj, down_proj):
    x = np.asarray(x)
    bsz, seqlen, dim = x.shape
    flat = np.ascontiguousarray(x.reshape(-1, dim), dtype=np.float32)
    T = flat.shape[0]
    gate_w = np.asarray(gate_w, dtype=np.float32)
    gate_proj = np.asarray(gate_proj, dtype=np.float32)
    up_proj = np.asarray(up_proj, dtype=np.float32)
    down_proj = np.asarray(down_proj, dtype=np.float32)

    top_val, top_idx = _route(flat, gate_w)

    idx_list = []
    cw_list = []
    for e in range(NUM_EXPERTS):
        mask = top_idx == e  # [T, K]
        tok = np.nonzero(mask.any(axis=1))[0]
        w = (top_val * mask).sum(axis=1)[tok].astype(np.float32)
        idx_list.append(tok)
        cw_list.append(w)

    max_cnt = max(len(i) for i in idx_list)
    tiles = _tiles_for(max_cnt)
    C = sum(tiles)
    nc = _get_program(tiles)

    flat16 = flat.astype(np.float16)
    XH = DT // 2
    in_maps = []
    for e in range(NUM_EXPERTS):
        tok = idx_list[e]
        cnt = len(tok)
        xfull = np.zeros((DIM, C), dtype=np.float16)
        xfull[:, :cnt] = flat16[tok].T
        # pack per tile as [half][p][a][token] — the SBUF image each x DMA
        # writes, so descriptors are fully contiguous
        parts = []
        o = 0
        for tt in tiles:
            seg = xfull[:, o : o + tt].reshape(2, XH, P, tt)
            parts.append(np.ascontiguousarray(seg.transpose(0, 2, 1, 3)).ravel())
            o += tt
        xtp = np.concatenate(parts)
        in_maps.append(
            {
                "xt": xtp,
                "wg": _pack_w(gate_proj[e].T),
                "wu": _pack_w(up_proj[e].T),
                "wd": _pack_w(down_proj[e].T),
            }
        )

    res = run_bass_kernel_spmd(nc, in_maps, core_ids=list(range(NUM_EXPERTS)))
    global LAST_RESULT
    LAST_RESULT = res

    out = np.zeros((T, DIM), dtype=np.float32)
    for e in range(NUM_EXPERTS):
        tok = idx_list[e]
        cnt = len(tok)
        if cnt:
            yt = np.asarray(res.results[e]["yt"][:, :cnt], dtype=np.float32)
            out[tok] += (yt * cw_list[e][None, :]).T
    return out.reshape(bsz, seqlen, dim)


# revision 29
# speedup vs baseline: 1.0043x; 1.0043x over previous
"""DeepseekV3 mini MoE MLP on 8 TRN2 NeuronCores.

Strategy: expert-parallel. The router (tiny: 0.1% of FLOPs) is computed
with jax ops that mirror the reference bit-for-bit; tokens are then
dispatched on the host to per-expert batches (the "all-to-all"), one
expert per NeuronCore. Each core runs a fused gate/up/silu/mul/down
kernel over its routed tokens. The combine (scatter-add weighted by the
top-k routing weights) happens on the host.

Matmul operands are fp16 (PE upconverts to FP22 internally, same
throughput as f32r but half the DMA traffic and SBUF footprint, and
fast-weight-load applies). PSUM accumulation stays fp32.

Layouts are feature-major ([dim, tokens]) so every matmul contracts over
the SBUF partition dim with no transposes anywhere on device. All DMAs
are shaped to be descriptor-friendly: x is packed on the host per token
tile as the exact SBUF image (128 fully-contiguous rows per transfer),
weights as [k_in(P), m_blk, k_blk, m_in] streamed in h-block-pair slabs.
Weight slabs + y writeback ride the Activation HWDGE ring; x tiles get
the SP ring to themselves. The down phase is software-pipelined one
token tile behind gate/up, so down weights aren't needed until ~75us in
and down matmuls never wait on the silu/mult chain. A few warm-up
matmuls on zeroed SBUF cover the first DMA's latency so the PE clock
(HAM) is already ramping before real work arrives.
"""

import numpy as np

import concourse.bass as bass
import concourse.mybir as mybir
import concourse.tile as tile
from concourse import bacc
from concourse.bass_utils import run_bass_kernel_spmd

DIM = 1024
HIDDEN = 1024
NUM_EXPERTS = 8
TOP_K = 2
P = 128
TT = 512  # main token tile (PSUM bank = 512 fp32)
DT = DIM // P  # 8 d-tiles
HT = HIDDEN // P  # 8 h-tiles
SW = 2  # h-blocks covered by the starter weight slab

F32 = mybir.dt.float32
F16 = mybir.dt.float16

_program_cache: dict[tuple, object] = {}
LAST_RESULT = None


def _build_expert_program(tiles: tuple):
    """One-expert FFN: yt = ((silu(x@wg.T) * (x@wu.T)) @ wd.T).T over C tokens.

    DRAM params (per core), all fp16:
      xt [DIM, C]          tokens, transposed (d-major)
      wg/wu [P, HT*DT*P]   gate/up proj packed [k_in, m_blk, k_blk, m_in]
      wd [P, HT*HT*P]      down proj packed    [k_in, m_blk, k_blk, m_in]
      yt [DIM, C]          output, transposed
    """
    C = sum(tiles)
    nc = bacc.Bacc(None, target_bir_lowering=False, debug=False)
    # xt is packed on the host per token tile as [half][p][a][token] — the
    # exact SBUF image — so every x DMA is 128 fully-contiguous descriptors.
    xt = nc.declare_dram_parameter("xt", [DIM * C], F16, isOutput=False)
    wg = nc.declare_dram_parameter("wg", [P, HT * DT * P], F16, isOutput=False)
    wu = nc.declare_dram_parameter("wu", [P, HT * DT * P], F16, isOutput=False)
    wd = nc.declare_dram_parameter("wd", [P, HT * HT * P], F16, isOutput=False)
    yt = nc.declare_dram_parameter("yt", [DIM, C], F16, isOutput=True)

    # gate/up weight slabs: pairs of h-blocks, streamed in consumption order
    # (wg01, wu01, wg23, ...), split across the two HWDGE rings.
    WCUTS = (0, 2, 4, 6, HT)
    NSCAL = 2  # slabs 0..1 ride the Activation ring, the rest ride SP

    with tile.TileContext(nc) as tc:
        with (
            tc.tile_pool(name="wpool", bufs=1) as wpool,
            tc.tile_pool(name="xpool", bufs=3) as xpool,
            tc.tile_pool(name="hpool", bufs=2) as hpool,
            tc.tile_pool(name="apool", bufs=3) as apool,
            tc.tile_pool(name="ypool", bufs=3) as ypool,
            tc.tile_pool(name="pg", bufs=2, space="PSUM") as pgpool,
            tc.tile_pool(name="pu", bufs=2, space="PSUM") as pupool,
            tc.tile_pool(name="py", bufs=2, space="PSUM") as pypool,
        ):
            wg_t, wu_t = [], []
            for i in range(len(WCUTS) - 1):
                n = (WCUTS[i + 1] - WCUTS[i]) * DT * P
                wg_t.append(wpool.tile([P, n], F16, name=f"wg{i}", tag=f"wg{i}"))
                wu_t.append(wpool.tile([P, n], F16, name=f"wu{i}", tag=f"wu{i}"))
            wd_a = wpool.tile([P, HT * HT * P], F16, name="wd_a", tag="wd_a")

            def w_at(tiles_, h, a):
                i = next(j for j in range(len(WCUTS) - 1) if h < WCUTS[j + 1])
                off = ((h - WCUTS[i]) * DT + a) * P
                return tiles_[i][:, off : off + P]

            # PE warm-up: a few matmuls on zeroed SBUF start the HAM busy
            # window while the first x/weight DMAs are still in flight.
            wz = wpool.tile([P, P], F16, name="wz", tag="wz")
            xz = wpool.tile([P, TT], F16, name="xz", tag="xz")
            nc.vector.memset(wz[:, :], 0.0)
            nc.vector.memset(xz[:, :], 0.0)
            pz = pgpool.tile([P, TT], F32, tag="pg")
            for _ in range(4):
                nc.tensor.matmul(pz[:, :], wz[:, :], xz[:, :], start=True, stop=True)

            XH = DT // 2  # d-blocks per x half-tile

            def load_x(t, tt, base, eng=None):
                eng = eng or nc.sync
                x_h = [
                    xpool.tile([P, XH * TT], F16, name=f"x{i}_{t}", tag=f"x{i}")
                    for i in range(2)
                ]
                for i in range(2):
                    lo = base + i * P * XH * tt
                    eng.dma_start(
                        out=x_h[i][:, :].rearrange("p (a t) -> p a t", a=XH)[:, :, :tt],
                        in_=xt.ap()[lo : lo + P * XH * tt].rearrange(
                            "(p a t) -> p a t", p=P, a=XH, t=tt
                        ),
                    )
                return x_h

            def gate_up(t, tt, x_h):
                h_sb = hpool.tile([P, HT * TT], F16, name=f"h_{t}", tag="h")
                for h in range(HT):
                    pg = pgpool.tile([P, tt], F32, tag="pg")
                    pu = pupool.tile([P, tt], F32, tag="pu")
                    for a in range(DT):
                        nc.tensor.matmul(
                            pg[:, :],
                            w_at(wg_t, h, a),
                            x_h[a // XH][:, (a % XH) * TT : (a % XH) * TT + tt],
                            start=(a == 0),
                            stop=(a == DT - 1),
                        )
                    for a in range(DT):
                        nc.tensor.matmul(
                            pu[:, :],
                            w_at(wu_t, h, a),
                            x_h[a // XH][:, (a % XH) * TT : (a % XH) * TT + tt],
                            start=(a == 0),
                            stop=(a == DT - 1),
                        )
                    act_sb = apool.tile([P, TT], F32, tag="act")
                    nc.scalar.activation(
                        act_sb[:, :tt], pg[:, :], mybir.ActivationFunctionType.Silu
                    )
                    nc.vector.tensor_tensor(
                        h_sb[:, h * TT : h * TT + tt],
                        act_sb[:, :tt],
                        pu[:, :],
                        mybir.AluOpType.mult,
                    )
                return h_sb

            def down(tt, ts, h_sb):
                for do in range(HT):
                    py = pypool.tile([P, tt], F32, tag="py")
                    for a in range(HT):
                        nc.tensor.matmul(
                            py[:, :],
                            wd_a[:, (do * HT + a) * P : (do * HT + a) * P + P],
                            h_sb[:, a * TT : a * TT + tt],
                            start=(a == 0),
                            stop=(a == HT - 1),
                        )
                    y_sb = ypool.tile([P, TT], F16, tag="y")
                    nc.vector.tensor_copy(y_sb[:, :tt], py[:, :])
                    nc.scalar.dma_start(
                        out=yt.ap()[do * P : (do + 1) * P, ts], in_=y_sb[:, :tt]
                    )

            # Software pipeline: gate/up for tile t, then down for tile t-1.
            # The down weights aren't needed until tile 1's gate/up is done
            # (~75us in), so the first-30us DMA rush is only x + gate/up
            # slabs. Down matmuls read an h_sb finished a full tile ago, so
            # they never wait on the silu/mult chain.
            prev = None  # (tt, ts, h_sb)
            x1_pre = None
            off = 0
            for t, tt in enumerate(tiles):
                ts = bass.ds(off, tt)
                if t == 1 and x1_pre is not None:
                    x_h = x1_pre
                else:
                    x_h = load_x(t, tt, DIM * off)
                off += tt
                if t == 0:
                    # Split the gate/up slab stream across both ~110GB/s
                    # rings so the last slab lands ~11us sooner: pairs 01/23
                    # on Activation, pairs 45/67 on SP behind x0. x1 rides
                    # Activation (SP is busy with the tail slabs); down
                    # weights follow on Activation — first needed ~75us in.
                    for i in range(len(WCUTS) - 1):
                        lo, hi = WCUTS[i] * DT * P, WCUTS[i + 1] * DT * P
                        eng = nc.scalar if i < NSCAL else nc.sync
                        eng.dma_start(out=wg_t[i][:, :], in_=wg.ap()[:, lo:hi])
                        eng.dma_start(out=wu_t[i][:, :], in_=wu.ap()[:, lo:hi])
                    if len(tiles) > 1:
                        x1_pre = load_x(1, tiles[1], DIM * off, eng=nc.scalar)
                    nc.scalar.dma_start(out=wd_a[:, :], in_=wd.ap()[:, :])
                h_sb = gate_up(t, tt, x_h)
                if prev is not None:
                    down(*prev)
                prev = (tt, ts, h_sb)
            down(*prev)
    nc.compile()
    return nc


def _tiles_for(max_cnt: int) -> tuple:
    """Token tiles covering max_cnt: a 384-token ramp tile (small so the
    first x DMA lands fast), full 512s, and a >=256 multiple-of-64 tail."""
    if max_cnt <= 384:
        return (max(256, ((max_cnt + 63) // 64) * 64),)
    rem = max_cnt - 384
    full, tail = divmod(rem, TT)
    tail = ((tail + 63) // 64) * 64
    if tail == 0:
        return (384,) + (TT,) * full
    if tail < 256:
        if full == 0:
            return (384, max(256, tail))
        full -= 1
        tail += TT  # in (512, 768): split into two >=256 tiles
        a = 256
        return (384,) + (TT,) * full + (a, tail - a)
    return (384,) + (TT,) * full + (tail,)


def _get_program(tiles: tuple):
    if tiles not in _program_cache:
        _program_cache[tiles] = _build_expert_program(tiles)
    return _program_cache[tiles]


def _pack_w(wt: np.ndarray) -> np.ndarray:
    """[K, M] weight (K contracted) -> [k_in(P), m_blk, k_blk, m_in] flattened
    to [P, M//P * K//P * P]; slab for output block mb is contiguous per row."""
    K, M = wt.shape
    return np.ascontiguousarray(
        wt.astype(np.float16)
        .reshape(K // P, P, M // P, P)
        .transpose(1, 2, 0, 3)
        .reshape(P, (M // P) * (K // P) * P)
    )


def _route(flat: np.ndarray, gate_w: np.ndarray):
    """Mirror the reference router bit-for-bit (jax ops, same backend)."""
    try:
        import jax
        import jax.numpy as jnp

        logits = jnp.asarray(flat) @ jnp.asarray(gate_w).T
        scores = jax.nn.sigmoid(logits)
        top_val, top_idx = jax.lax.top_k(scores, TOP_K)
        top_val = top_val / (top_val.sum(-1, keepdims=True) + 1e-9)
        return np.asarray(top_val), np.asarray(top_idx)
    except Exception:
        # numpy fallback: identical selection semantics (stable descending)
        logits = flat @ gate_w.T
        scores = 1.0 / (1.0 + np.exp(-logits))
        order = np.argsort(-scores, axis=-1, kind="stable")
        top_idx = order[:, :TOP_K].astype(np.int32)
        top_val = np.take_along_axis(scores, top_idx, axis=-1)
        top_val = top_val / (top_val.sum(-1, keepdims=True) + 1e-9)
        return top_val.astype(np.float32), top_idx


def kernel(x, gate_w, gate_proj, up_proj, down_proj):
    x = np.asarray(x)
    bsz, seqlen, dim = x.shape
    flat = np.ascontiguousarray(x.reshape(-1, dim), dtype=np.float32)
    T = flat.shape[0]
    gate_w = np.asarray(gate_w, dtype=np.float32)
    gate_proj = np.asarray(gate_proj, dtype=np.float32)
    up_proj = np.asarray(up_proj, dtype=np.float32)
    down_proj = np.asarray(down_proj, dtype=np.float32)

    top_val, top_idx = _route(flat, gate_w)

    idx_list = []
    cw_list = []
    for e in range(NUM_EXPERTS):
        mask = top_idx == e  # [T, K]
        tok = np.nonzero(mask.any(axis=1))[0]
        w = (top_val * mask).sum(axis=1)[tok].astype(np.float32)
        idx_list.append(tok)
        cw_list.append(w)

    max_cnt = max(len(i) for i in idx_list)
    tiles = _tiles_for(max_cnt)
    C = sum(tiles)
    nc = _get_program(tiles)

    flat16 = flat.astype(np.float16)
    XH = DT // 2
    in_maps = []
    for e in range(NUM_EXPERTS):
        tok = idx_list[e]
        cnt = len(tok)
        xfull = np.zeros((DIM, C), dtype=np.float16)
        xfull[:, :cnt] = flat16[tok].T
        # pack per tile as [half][p][a][token] — the SBUF image each x DMA
        # writes, so descriptors are fully contiguous
        parts = []
        o = 0
        for tt in tiles:
            seg = xfull[:, o : o + tt].reshape(2, XH, P, tt)
            parts.append(np.ascontiguousarray(seg.transpose(0, 2, 1, 3)).ravel())
            o += tt
        xtp = np.concatenate(parts)
        in_maps.append(
            {
                "xt": xtp,
                "wg": _pack_w(gate_proj[e].T),
                "wu": _pack_w(up_proj[e].T),
                "wd": _pack_w(down_proj[e].T),
            }
        )

    res = run_bass_kernel_spmd(nc, in_maps, core_ids=list(range(NUM_EXPERTS)))
    global LAST_RESULT
    LAST_RESULT = res

    out = np.zeros((T, DIM), dtype=np.float32)
    for e in range(NUM_EXPERTS):
        tok = idx_list[e]
        cnt = len(tok)
        if cnt:
            yt = np.asarray(res.results[e]["yt"][:, :cnt], dtype=np.float32)
            out[tok] += (yt * cw_list[e][None, :]).T
    return out.reshape(bsz, seqlen, dim)
